# revision 61
# baseline (speedup 1.0000x reference)
"""Trainium2 Bass kernel for nn_Chambers (6-tower MLP + coupled sigmoid recurrence).

Data-parallel over 8 NeuronCores; each core runs 16 chunks of 1024 samples.
res is transposed + bf16-cast host-side (row 100 = ones so b1 rides the W1
lhsT), removing all PE transposes. The four MLP layers run as bf16 matmuls
(chambers packed block-diagonally); L4 accumulates all 16 chunks into one
persistent [96,1024] PSUM tile via per-chunk W4 column stacks, so raw needs
no per-chunk engine copies.

Activation work is split across engines to beat the single-ACT roofline:
ACT does the L1 silus (exact, 6/chunk) + the L2 pair-2 tile + the L3 ch4/5
tile; DVE+Pool evaluate the remaining silus (L2 pairs 0/1, L3 ch0-3) with a
degree-3 odd-tanh polynomial (max err ~5e-4 on the observed pre-activation
range) as a 5-op pipeline: psum->bf16 affine [DVE], square [Pool], affine
[DVE 4x], two multiplies [DVE 2x]. The chunk loop is software-pipelined
with a 1-chunk skew (L3/L4 of chunk i-1 run during chunk i) so the Pool
round-trip never bubbles the DVE queue, and L1 of chunk i+1 is interleaved
into chunk i so ACT's silu run restarts early. The coupled sigmoid
recurrence runs as 4 independent 256-column chains of block-diagonal bf16
matmuls with b4 folded into an ones-row of the raw operand; raw_out gets
b4 added host-side.

Sync discipline (walrus: <=1 sem wait per instruction): cross-engine deps
are pre-observed by zero-cost ldweights "touches" on PE (all PE-read tiles
are bf16) and 1-element copies on ACT/DVE/Pool; tile-slot WAW hazards are
absorbed by nops carrying the wait; a post-pass drops sem waits already
covered by an earlier wait on the same queue.
"""
import numpy as np
import ml_dtypes

import concourse.bass as bass
import concourse.mybir as mybir
from concourse.bass_utils import run_bass_kernel_spmd
from concourse.tile import TileContext
from concourse.tile_scheduler import N_PROCS
from concourse.vector_clock import ScopedClock
from bass_rust import add_dep_helper

F32 = mybir.dt.float32
BF16 = mybir.dt.bfloat16
AF = mybir.ActivationFunctionType
ALU = mybir.AluOpType
bfdt = ml_dtypes.bfloat16

B = 131072
NCORES = 8
BS = B // NCORES           # 16384 samples per core
T = 1024                   # chunk (samples)
NCH = BS // T              # 16 chunks
RES_DIM = 100
CF_ITERS = 5
CF_K = 0.02

# silu(x) ~= 0.5x + x^2*(c0 + c1*x^2), minimax-fit per layer input range
C0_L2, C1_L2 = 0.24709027, -0.01595315     # range ±1.45, err 5.1e-4
C0_L3 = 0.24992208                         # D1 on ±0.55, err ~1e-3

# wb (bf16) column layout
W1C = 0                    # 6*128, rows 0:101 (row 100 = b1)
W2EC = W1C + 6 * 128       # 3*64  even chambers, out rows 0:64
W2OC = W2EC + 3 * 64       # 3*128 odd chambers -> out rows 64:128 (cols 0:64 zero)
W3PC = W2OC + 3 * 128      # 128   pairs 0/1 merged: ch2/3 -> rows 64:128
W3EC = W3PC + 128          # 64    ch0/1 -> rows 0:64
W3YC = W3EC + 64           # 64    ch4/5 -> rows 0:64 (used at out base 0 and 64)
W4AC = W3YC + 64           # 16*96 per-chunk stacks, chambers 0-3 (rows 0:128)
W4BC = W4AC + 16 * 96      # 16*96 chambers 4-5; rows 0:64 and dup at 64:128
CDC = W4BC + 16 * 96       # 96    block-diag decay*coupling*k (16 groups)
I97C = CDC + 96            # 96    rows 0:96 identity, row 96 = b4 tiled
WBCOLS = I97C + 96

# wf (f32) column layout (per-partition bias packs)
BYC = 0     # Y silu bias (b3 ch4/5 by 32s)
B2HC = 1    # 3 cols: b2 pair packs / 2 (DVE pass1)
B2FC = 4    # 3 cols: b2 pair packs (ACT silu)
B3AHC = 7   # L3A pack: b3[c]/2 by 32s
B4C = 8     # sigmoid bias: b4 tiled over 96 rows
B3AF = 9    # L3A full bias pack (ACT silu, last chunk)
FCOLS = 10


class TC(TileContext):
    """TileContext with a walrus-compatible epilogue (split final waits)."""

    def _drain_and_barrier(self, tick_clock, wait_clock):
        nc = self.nc
        full = ScopedClock({None: tick_clock.global_clock})
        for scope, vc in full.items():
            for proc in range(N_PROCS):
                t = vc.peek_next(proc) - 1
                if t > 0:
                    sc = ScopedClock()
                    sc.require_at_least(scope, proc, t)
                    w = nc.sync.nop(nofuse=True)
                    wait_clock.add_sem_waits(w.ins, sc)
        for eng in nc.engines.values():
            eng.drain(fusable=False)
        nc.all_engine_barrier(sem_only=True)
        assert self.sems is not None
        popped = nc._tile_sem_poison_stack.pop()
        assert popped is self._sem_poison
        nc.clear_and_free_semaphores(list(self.sems.allocated().values()))
        for eng in nc.engines.values():
            eng.drain(fusable=False)
        nc.all_engine_barrier(sem_only=True)


def _order(after_inst, before_inst):
    if after_inst is not None and before_inst is not None:
        add_dep_helper(after_inst.ins, before_inst.ins, sync=False, reason="order")


def _drop_covered_waits(nc):
    """Remove sem waits already guaranteed by an earlier instruction on the
    same engine queue waiting the same semaphore at >= value (sem values are
    monotone, so the later wait is redundant). Brings every instruction
    within walrus's 1-wait limit."""
    import bass_rust
    import re
    lane = re.compile(r"^(PE|Activation|DVE|Pool|SP)_\d+$")
    for fn in nc.m.functions:
        seen = {}
        for blk in fn.blocks:
            for ins in blk.instructions:
                si = ins.sync_info
                if si is None or not si.on_wait:
                    continue
                eng = ins.engine
                cov = seen.setdefault(eng, {})
                keep = []
                for w in si.on_wait:
                    key = (w.sync_type, w.id)
                    if (w.wait_mode == "sem-ge-imm"
                            and w.ant_name and lane.match(w.ant_name)
                            and cov.get(key, -1) >= w.wait_value):
                        continue
                    keep.append(w)
                for w in si.on_wait:
                    key = (w.sync_type, w.id)
                    if (w.wait_mode == "sem-ge-imm"
                            and w.ant_name and lane.match(w.ant_name)):
                        cov[key] = max(cov.get(key, -1), w.wait_value)
                if len(keep) != len(si.on_wait):
                    ins.sync_info = bass_rust.SyncInfo(
                        on_wait=keep, on_update=list(si.on_update))


def build_module():
    nc = bass.Bass()
    resT_d = nc.dram_tensor("resT", [RES_DIM + 1, BS], BF16, kind="ExternalInput")
    wb_d = nc.dram_tensor("wb", [128, WBCOLS], BF16, kind="ExternalInput")
    wf_d = nc.dram_tensor("wf", [128, FCOLS], F32, kind="ExternalInput")
    raw_d = nc.dram_tensor("raw_out", [96, T], F32, kind="ExternalOutput")
    act_d = nc.dram_tensor("act_out", [96, T], BF16, kind="ExternalOutput")

    with TC(nc) as tc:
        with (
            tc.tile_pool(name="wconst", bufs=1) as wpool,
            tc.tile_pool(name="sbh", bufs=2) as sbh,
            tc.tile_pool(name="sbrec", bufs=1) as sbrec,
            tc.tile_pool(name="psmm", bufs=3, space="PSUM") as psmm,
            tc.tile_pool(name="psl4", bufs=1, space="PSUM") as psl4,
        ):
            # ---- DMAs: W1 block + chunk-0 res first so compute starts early
            wb = wpool.tile([128, WBCOLS], BF16)
            resT = wpool.tile([RES_DIM + 1, BS], BF16)
            wf = wpool.tile([128, FCOLS], F32)
            nc.sync.dma_start(out=wb[:, 0:2 * 128], in_=wb_d[:, 0:2 * 128])
            nc.sync.dma_start(out=resT[:, 0:T], in_=resT_d[:, 0:T])
            nc.sync.dma_start(out=wb[:, 2 * 128:W2EC], in_=wb_d[:, 2 * 128:W2EC])
            nc.sync.dma_start(out=wf[:], in_=wf_d[:])
            nc.sync.dma_start(out=wb[:, W2EC:], in_=wb_d[:, W2EC:])
            nc.sync.dma_start(out=resT[:, T:4 * T], in_=resT_d[:, T:4 * T])
            nc.sync.dma_start(out=resT[:, 4 * T:], in_=resT_d[:, 4 * T:])

            raw_f = sbrec.tile([96, T], F32)
            raw_b = sbrec.tile([97, T], BF16)  # row 96 = ones (b4 via I97 pack)
            act_r = sbrec.tile([96, T], BF16)
            act_o = sbrec.tile([96, T], BF16)
            scrA = sbrec.tile([1, 64], F32)
            scrD = sbrec.tile([1, 64], F32)
            scrP = sbrec.tile([1, 64], F32)
            nc.vector.memset(raw_b[96:97, :], 1.0)

            l4p = psl4.tile([128, T], F32)

            # ---- engine tails + touch helpers
            pe_tail = None
            act_tail = None
            dve_tail = None
            gp_tail = None

            def pe_touch(src_ap):
                """ldweights touch: observes src's producer on PE, costs 0."""
                nonlocal pe_tail
                w = nc.tensor.ldweights(src_ap)
                _order(w, pe_tail)
                pe_tail = w
                return w

            acol = [0]

            def act_touch(src_ap):
                nonlocal act_tail
                t = acol[0] % 64
                acol[0] += 1
                s = nc.scalar.activation(scrA[0:1, t:t + 1], src_ap, AF.Copy)
                _order(s, act_tail)
                act_tail = s
                return s

            dcol = [0]

            def dve_touch(src_ap):
                nonlocal dve_tail
                t = dcol[0] % 64
                dcol[0] += 1
                c = nc.vector.tensor_copy(scrD[0:1, t:t + 1], src_ap)
                _order(c, dve_tail)
                dve_tail = c
                return c

            pcol = [0]

            def gp_touch(src_ap):
                nonlocal gp_tail
                t = pcol[0] % 64
                pcol[0] += 1
                c = nc.gpsimd.tensor_copy(scrP[0:1, t:t + 1], src_ap)
                _order(c, gp_tail)
                gp_tail = c
                return c

            def mm(out_ap, lhs_ap, rhs_ap, **kw):
                nonlocal pe_tail
                m = nc.tensor.matmul(out_ap, lhs_ap, rhs_ap, **kw)
                _order(m, pe_tail)
                pe_tail = m
                return m

            def act_op(emit):
                nonlocal act_tail
                s = emit()
                _order(s, act_tail)
                act_tail = s
                return s

            def dve_op(emit):
                nonlocal dve_tail
                s = emit()
                _order(s, dve_tail)
                dve_tail = s
                return s

            def gp_op(emit):
                nonlocal gp_tail
                s = emit()
                _order(s, gp_tail)
                gp_tail = s
                return s

            # ---- same-engine/cross-engine WAW absorbers: a slot-reusing
            # write would carry a second sem wait (engine write-acks are
            # pipelined, so queue order alone doesn't cover WAW); a nop
            # takes that wait instead.
            def act_absorb(dep):
                nonlocal act_tail
                n = nc.scalar.nop(nofuse=True)
                add_dep_helper(n.ins, dep.ins, sync=True, reason="waw")
                _order(n, act_tail)
                act_tail = n

            def dve_absorb(dep):
                nonlocal dve_tail
                n = nc.vector.nop(nofuse=True)
                add_dep_helper(n.ins, dep.ins, sync=True, reason="waw")
                _order(n, dve_tail)
                dve_tail = n

            def pe_absorb(dep):
                nonlocal pe_tail
                w = nc.tensor.ldweights(wb[0:1, 0:2])
                add_dep_helper(w.ins, dep.ins, sync=True, reason="waw")
                _order(w, pe_tail)
                pe_tail = w

            writers = {}

            def slot_guard(tag, bufs, absorb_fn):
                # hazard distance is bufs or bufs-1 depending on dynamic slot
                # assignment; absorb both candidates (writers may sit on
                # different engines when a tag is served by ACT and DVE).
                lst = writers.setdefault(tag, [])
                d = max(1, bufs - 1)
                done = []
                for dist in (d + 1, d, max(1, d - 1)):
                    if len(lst) >= dist and not any(lst[-dist] is x for x in done):
                        done.append(lst[-dist])
                        absorb_fn(lst[-dist])

            def slot_record(tag, inst):
                writers.setdefault(tag, []).append(inst)

            # ---- psum tag rotation: 3 [128,1024] slots; before reusing a
            # slot, PE pre-observes the output of the op that drained it.
            tag_rr = [0]
            tag_state = [None, None, None]

            def new_mm_tile(name, width=T):
                tg = tag_rr[0] % 3
                tag_rr[0] += 1
                st = tag_state[tg]
                if st is not None:
                    pe_touch(st)
                    tag_state[tg] = None
                t = psmm.tile([128, width], F32, tag=f"mm{tg}", bufs=1, name=name)
                return t, tg

            def mark(tg, out_tile_ap):
                tag_state[tg] = out_tile_ap

            # ---- DVE/Pool approx-silu pipeline, software-pipelined --------
            # start: pass1 (DVE, psum->bf16) + square (Pool). finish: q/v/out
            # (DVE). Finishes lag starts by DVE_LOOKAHEAD tiles so Pool's
            # square overlaps DVE work instead of bubbling the DVE queue.
            ptouch_cells = []
            dve_pending = []
            DVE_LOOKAHEAD = 2

            def silu_dve_start(pm, bh_col, h_out, c0, c1, degree3, sc_pool,
                               htag, hbufs, u_on_dve=False):
                y = sc_pool.tile([128, T], BF16, tag="sy", bufs=6, name="sy")
                u = sc_pool.tile([128, T], BF16, tag="su", bufs=6, name="su")
                # y-slot WAR: before pass1 rewrites y[k-6]'s slot, DVE
                # observes the Pool scratch cell written before u[k-5] --
                # implying Pool finished reading y[k-6]. Cells are never
                # reused, so no tile lifetime is extended.
                k = len(ptouch_cells)
                if k >= 5:
                    c_ = ptouch_cells[k - 5]
                    dve_touch(scrP[0:1, c_:c_ + 1])
                dve_op(lambda: nc.vector.tensor_scalar(
                    y[:], pm[:], 0.5, wf[:, bh_col:bh_col + 1], ALU.mult, ALU.add))
                ptouch_cells.append(pcol[0] % 64)
                if u_on_dve:
                    dve_op(lambda: nc.vector.tensor_tensor(u[:], y[:], y[:], ALU.mult))
                else:
                    gp_touch(y[0:1, 0:1])
                    gp_op(lambda: nc.gpsimd.tensor_tensor(u[:], y[:], y[:], ALU.mult))
                dve_pending.append((y, u, h_out, c0, c1, degree3, sc_pool,
                                    htag, hbufs, u_on_dve))
                return y

            def dve_finish_one():
                (y, u, h_out, c0, c1, degree3, sc_pool,
                 htag, hbufs, u_on_dve) = dve_pending.pop(0)
                if not u_on_dve:
                    dve_touch(u[0:1, 0:1])
                slot_guard(htag, hbufs, dve_absorb)
                if degree3:
                    q = sc_pool.tile([128, T], BF16, tag="sq", bufs=2, name="sq")
                    v = sc_pool.tile([128, T], BF16, tag="sv", bufs=2, name="sv")
                    dve_op(lambda: nc.vector.tensor_scalar(
                        q[:], u[:], 16.0 * c1, 4.0 * c0, ALU.mult, ALU.add))
                    dve_op(lambda: nc.vector.tensor_tensor(v[:], u[:], q[:], ALU.mult))
                    w_ = dve_op(lambda: nc.vector.tensor_tensor(h_out[:], v[:], y[:], ALU.add))
                else:
                    v = sc_pool.tile([128, T], BF16, tag="sv", bufs=2, name="sv")
                    dve_op(lambda: nc.vector.tensor_scalar(
                        v[:], u[:], 4.0 * c0, None, ALU.mult))
                    w_ = dve_op(lambda: nc.vector.tensor_tensor(h_out[:], v[:], y[:], ALU.add))
                slot_record(htag, w_)

            finished_labels = set()

            def finish_until(label):
                if label in finished_labels or label not in started_labels:
                    return
                while pending_labels:
                    lb = pending_labels.pop(0)
                    dve_finish_one()
                    finished_labels.add(lb)
                    if lb == label:
                        return
                raise AssertionError(f"label {label} not pending")

            pending_labels = []

            started_labels = set()

            def silu_start(label, pm, bh_col, h_out, c0, c1, degree3,
                           htag, hbufs, u_on_dve=False):
                started_labels.add(label)
                y = silu_dve_start(pm, bh_col, h_out, c0, c1, degree3, sbh,
                                   htag, hbufs, u_on_dve)
                pending_labels.append(label)
                return y

            # ---- startup observation: each engine sees the DMAs it needs
            pe_touch(wb[0:1, 0:2])            # W1 block lane
            pe_touch(resT[0:1, 0:2])          # res chunk 0 lane
            act_touch(wf[0:1, BYC:BYC + 1])   # wf lane for ACT biases
            dve_touch(wf[0:1, B2HC:B2HC + 1])  # wf lane for DVE biases
            seen_wbrest = [False]
            seen_resB = [False]
            seen_resC = [False]

            # Pipeline skew: chunk i emits L1[i]+L2[i], then L3[i-1] (whose
            # DVE silus got a full chunk of Pool overlap), then L4[i-2].
            def emit_l3_pa(j, h2s):
                """L3 chambers 0-3 for chunk j; pr0/pr1 finishes must be done."""
                pe_touch(h2s[1][0:1, 0:2])
                pa, tga = new_mm_tile("pm3a")
                for s in range(2):
                    mm(pa[:, s * 512:(s + 1) * 512],
                       wb[:, W3PC:W3PC + 128],
                       h2s[1][:, s * 512:(s + 1) * 512], start=True, stop=False)
                    mm(pa[0:64, s * 512:(s + 1) * 512],
                       wb[:, W3EC:W3EC + 64],
                       h2s[0][:, s * 512:(s + 1) * 512], start=False, stop=True)
                h3a = sbh.tile([128, T], BF16, tag="h3a", bufs=3, name="h3a")
                if j >= NCH - 2:
                    slot_guard("h3a", 3, act_absorb)
                    w_ = act_op(lambda: nc.scalar.activation(
                        h3a[:], pa[:], AF.Silu, bias=wf[:, B3AF:B3AF + 1]))
                    slot_record("h3a", w_)
                    mark(tga, h3a[0:1, 0:2])
                else:
                    y3 = silu_start(("l3a", j), pa, B3AHC, h3a, C0_L3, 0.0,
                                    False, "h3a", 3)
                    mark(tga, y3[0:1, 0:2])
                return h3a

            def emit_l3_y(j, h2s):
                """L3 chambers 4/5 for chunk j; pr2 finish must be done."""
                pe_touch(h2s[2][0:1, 0:2])
                py, tgy = new_mm_tile("pm3y", width=512)
                mm(py[0:64, 0:512], wb[:, W3YC:W3YC + 64],
                   h2s[2][:, 0:512], start=True, stop=True)
                mm(py[64:128, 0:512], wb[:, W3YC:W3YC + 64],
                   h2s[2][:, 512:1024], start=True, stop=True)
                h3y = sbh.tile([128, 512], BF16, tag="h3y", bufs=3, name="h3y")
                slot_guard("h3y", 3, act_absorb)
                w_ = act_op(lambda py=py, h3y=h3y: nc.scalar.activation(
                    h3y[:], py[:], AF.Silu, bias=wf[:, BYC:BYC + 1]))
                slot_record("h3y", w_)
                mark(tgy, h3y[0:1, 0:2])
                return h3y

            def emit_l3(j, h2s):
                return emit_l3_pa(j, h2s), emit_l3_y(j, h2s)

            def emit_l4(j, h3a, h3y):
                """L4 for chunk j into the persistent psum; finish l3a[j] first."""
                finish_until(("l3a", j))
                pe_touch(h3a[0:1, 0:2])
                for s in range(2):
                    mm(l4p[0:96, s * 512:(s + 1) * 512],
                       wb[:, W4AC + 96 * j:W4AC + 96 * (j + 1)],
                       h3a[:, s * 512:(s + 1) * 512],
                       start=(j == 0), stop=False)
                pe_touch(h3y[0:1, 0:2])
                mm(l4p[0:96, 0:512],
                   wb[0:64, W4BC + 96 * j:W4BC + 96 * (j + 1)],
                   h3y[0:64, 0:512], start=False, stop=(j == NCH - 1))
                return mm(l4p[0:96, 512:1024],
                   wb[64:128, W4BC + 96 * j:W4BC + 96 * (j + 1)],
                   h3y[64:128, 0:512], start=False, stop=(j == NCH - 1))

            def emit_l1_chamber(j, c, h1s):
                """One L1 chamber for chunk j (mms + ACT silu)."""
                co = j * T
                if j == 1 and not seen_resB[0]:
                    pe_touch(resT[0:1, T:T + 2])
                    seen_resB[0] = True
                if j == 4 and not seen_resC[0]:
                    pe_touch(resT[0:1, 4 * T:4 * T + 2])
                    seen_resC[0] = True
                pm, tg = new_mm_tile(f"pm1_{c}")
                for s in range(2):
                    mm(pm[:, s * 512:(s + 1) * 512],
                       wb[0:RES_DIM + 1, W1C + c * 128:W1C + (c + 1) * 128],
                       resT[:, co + s * 512:co + (s + 1) * 512],
                       start=True, stop=True)
                h1 = sbh.tile([128, T], BF16, tag="h1", bufs=7, name="h1")
                slot_guard("h1", 7, act_absorb)
                w_ = act_op(lambda pm=pm, h1=h1: nc.scalar.activation(
                    h1[:], pm[:], AF.Silu))
                slot_record("h1", w_)
                mark(tg, h1[0:1, 0:2])
                h1s.append(h1)

            def emit_l1(j):
                h1s = []
                for c in range(6):
                    emit_l1_chamber(j, c, h1s)
                return h1s

            prev_l2 = None   # (i-1, h2s, last_pr_label)
            prev_l3 = None   # (i-2, h3a, h3y)

            h1s = emit_l1(0)
            pe_touch(wb[0:1, W2EC * 2:W2EC * 2 + 2])

            for i in range(NCH):
                # ---- DVE finishes for the previous chunk first: their Pool
                # squares completed during the last chunk, and L3[i-1]'s PE
                # matmuls (emitted below) wait on them.
                if prev_l2 is not None and prev_l2[2] is not None:
                    finish_until(prev_l2[2])

                # ---- L2: 3 pair tiles, interleaved with the previous
                # chunk's L3/L4 matmuls; L1[i+1] at the end so ACT's next
                # chunk starts as soon as its own queue drains.
                last_chunk = i == NCH - 1
                h2s = []
                next_h1s = []
                last_pr_label = None
                nh3a = nh3y = None
                for pr in range(3):
                    pe_touch(h1s[2 * pr + 1][0:1, 0:2])
                    pm2, tg2 = new_mm_tile(f"pm2_{pr}")
                    for s in range(2):
                        mm(pm2[:, s * 512:(s + 1) * 512],
                           wb[:, W2OC + pr * 128:W2OC + (pr + 1) * 128],
                           h1s[2 * pr + 1][:, s * 512:(s + 1) * 512],
                           start=True, stop=False)
                        mm(pm2[0:64, s * 512:(s + 1) * 512],
                           wb[:, W2EC + pr * 64:W2EC + (pr + 1) * 64],
                           h1s[2 * pr][:, s * 512:(s + 1) * 512],
                           start=False, stop=True)
                    h2 = sbh.tile([128, T], BF16, tag="h2", bufs=7, name="h2")
                    on_act = (pr == 2) or last_chunk
                    if on_act:
                        slot_guard("h2", 7, act_absorb)
                        w_ = act_op(lambda pm2=pm2, h2=h2, pr=pr: nc.scalar.activation(
                            h2[:], pm2[:], AF.Silu,
                            bias=wf[:, B2FC + pr:B2FC + pr + 1]))
                        slot_record("h2", w_)
                        mark(tg2, h2[0:1, 0:2])
                    else:
                        last_pr_label = ("pr", i, pr)
                        y = silu_start(last_pr_label, pm2, B2HC + pr,
                                       h2, C0_L2, C1_L2, True, "h2", 7)
                        mark(tg2, y[0:1, 0:2])
                    h2s.append(h2)
                    # next chunk's L1 chambers slot in here so ACT's silu
                    # run for chunk i+1 starts as early as possible
                    if not last_chunk:
                        emit_l1_chamber(i + 1, 2 * pr, next_h1s)
                        emit_l1_chamber(i + 1, 2 * pr + 1, next_h1s)
                    if prev_l2 is not None:
                        if pr == 0:
                            nh3a = emit_l3_pa(prev_l2[0], prev_l2[1])
                        elif pr == 1:
                            nh3y = emit_l3_y(prev_l2[0], prev_l2[1])
                        elif prev_l3 is not None:
                            emit_l4(*prev_l3)

                if prev_l2 is not None:
                    prev_l3 = (prev_l2[0], nh3a, nh3y)
                prev_l2 = (i, h2s, last_pr_label)
                h1s = next_h1s

            # ---- drain the skewed tail
            j, ph2s, plabel = prev_l2
            if plabel is not None:
                finish_until(plabel)
            nh3 = emit_l3(j, ph2s)
            emit_l4(*prev_l3)
            last_mm = emit_l4(j, *nh3)
            for _ in range(3):
                slot_record("recmm", last_mm)

            # ---- coupled sigmoid recurrence on [96, T], 4 independent
            # column chains to cut the serial mm->sigmoid latency ----------
            NQ, QW = 4, T // 4
            cp1 = dve_op(lambda: nc.vector.tensor_copy(raw_f[:], l4p[0:96, :]))
            cp2 = dve_op(lambda: nc.vector.tensor_copy(raw_b[0:96, :], l4p[0:96, :]))
            act_absorb(cp2)
            sig = None
            for q in range(NQ):
                sig = act_op(lambda q=q: nc.scalar.activation(
                    act_r[:, q * QW:(q + 1) * QW], l4p[0:96, q * QW:(q + 1) * QW],
                    AF.Sigmoid, bias=wf[0:96, B4C:B4C + 1]))
                slot_record(f"recact{q}", sig)
            pe_touch(raw_b[0:1, 0:2])
            for kk in range(CF_ITERS):
                last = kk == CF_ITERS - 1
                for q in range(NQ):
                    pe_touch(act_r[0:1, q * QW:q * QW + 2])
                    pm5, tg5 = new_mm_tile("pm5", width=QW)
                    mm(pm5[0:96, 0:QW],
                       wb[0:96, CDC:CDC + 96],
                       act_r[:, q * QW:(q + 1) * QW], start=True, stop=False)
                    w_ = mm(pm5[0:96, 0:QW],
                       wb[0:97, I97C:I97C + 96],
                       raw_b[:, q * QW:(q + 1) * QW], start=False, stop=True)
                    slot_record("recmm", w_)
                    slot_guard(f"recact{q}", 1, act_absorb)
                    dst = act_o if last else act_r
                    sg = act_op(lambda pm5=pm5, q=q, dst=dst: nc.scalar.activation(
                        dst[:, q * QW:(q + 1) * QW], pm5[0:96, 0:QW],
                        AF.Sigmoid))
                    mark(tg5, dst[0:1, q * QW:q * QW + 2])
                    slot_record(f"recact{q}", sg)
                    sig = sg

            n1 = nc.sync.nop(nofuse=True)
            add_dep_helper(n1.ins, cp1.ins, sync=True, reason="dma-absorb")
            nc.sync.dma_start(out=raw_d[:], in_=raw_f[:])
            n2 = nc.sync.nop(nofuse=True)
            add_dep_helper(n2.ins, sg.ins, sync=True, reason="dma-absorb")
            _order(n2, n1)
            nc.sync.dma_start(out=act_d[:], in_=act_o[:])

    _drop_covered_waits(nc)
    return nc


def _pack_consts(W1, b1, W2, b2, W3, b3, W4, b4, coupling, decay):
    wb = np.zeros((128, WBCOLS), dtype=np.float32)
    for c in range(6):
        wb[0:RES_DIM, W1C + c * 128:W1C + (c + 1) * 128] = W1[c]
        wb[RES_DIM, W1C + c * 128:W1C + (c + 1) * 128] = b1[c]
    for pr in range(3):
        wb[:, W2EC + pr * 64:W2EC + (pr + 1) * 64] = W2[2 * pr]
        wb[:, W2OC + pr * 128 + 64:W2OC + (pr + 1) * 128] = W2[2 * pr + 1]
    # L3 pairs 0/1 merged: ch0/1 -> rows 0:64 (W3EC), ch2/3 -> rows 64:128
    wb[0:64, W3EC:W3EC + 32] = W3[0]
    wb[64:128, W3EC + 32:W3EC + 64] = W3[1]
    wb[0:64, W3PC + 64:W3PC + 96] = W3[2]
    wb[64:128, W3PC + 96:W3PC + 128] = W3[3]
    # Y: ch4/5; same lhsT used at out rows 0:64 (cols 0:512) and 64:128
    wb[0:64, W3YC:W3YC + 32] = W3[4]
    wb[64:128, W3YC + 32:W3YC + 64] = W3[5]
    for i in range(NCH):
        for c in range(4):
            wb[32 * c:32 * (c + 1), W4AC + 96 * i + 6 * i + c] = W4[c]
        for c2 in range(2):
            wb[32 * c2:32 * (c2 + 1), W4BC + 96 * i + 6 * i + 4 + c2] = W4[4 + c2]
            wb[64 + 32 * c2:64 + 32 * (c2 + 1),
               W4BC + 96 * i + 6 * i + 4 + c2] = W4[4 + c2]
    cd = (decay[:, None] * coupling * CF_K).astype(np.float32)
    for g in range(NCH):
        wb[6 * g:6 * g + 6, CDC + 6 * g:CDC + 6 * g + 6] = cd
    wb[0:96, I97C:I97C + 96] = np.eye(96, dtype=np.float32)
    wb[96, I97C:I97C + 96] = np.tile(b4, NCH)

    wf = np.zeros((128, FCOLS), dtype=np.float32)
    for k in range(4):
        wf[32 * k:32 * (k + 1), BYC] = b3[4 + (k % 2)]
    for pr in range(3):
        wf[0:64, B2HC + pr] = b2[2 * pr] / 2
        wf[64:128, B2HC + pr] = b2[2 * pr + 1] / 2
        wf[0:64, B2FC + pr] = b2[2 * pr]
        wf[64:128, B2FC + pr] = b2[2 * pr + 1]
    for c in range(4):
        wf[32 * c:32 * (c + 1), B3AHC] = b3[c] / 2
        wf[32 * c:32 * (c + 1), B3AF] = b3[c]
    wf[0:96, B4C] = np.tile(b4, NCH)
    return wb.astype(bfdt), wf


def kernel(res, W1, b1, W2, b2, W3, b3, W4, b4, coupling, decay):
    res = np.asarray(res, dtype=np.float32)
    args = [np.asarray(a, dtype=np.float32)
            for a in (W1, b1, W2, b2, W3, b3, W4, b4, coupling, decay)]
    wb, wf = _pack_consts(*args)
    b4f = args[7]

    nc = build_module()
    in_maps = []
    for i in range(NCORES):
        shard = res[i * BS:(i + 1) * BS]
        rt = np.empty((RES_DIM + 1, BS), dtype=bfdt)
        rt[0:RES_DIM] = shard.T.astype(bfdt)
        rt[RES_DIM] = bfdt(1.0)
        in_maps.append({"resT": rt, "wb": wb, "wf": wf})
    results = run_bass_kernel_spmd(nc, in_maps, core_ids=list(range(NCORES)))

    acts, raws = [], []
    for r in results.results:
        a = np.asarray(r["act_out"], dtype=np.float32)
        w = np.asarray(r["raw_out"], dtype=np.float32)
        acts.append(a.reshape(NCH, 6, T).transpose(0, 2, 1).reshape(BS, 6))
        raw = w.reshape(NCH, 6, T).transpose(0, 2, 1).reshape(BS, 6) + b4f
        raws.append(raw)
    return np.concatenate(acts, 0), np.concatenate(raws, 0)
